# revision 1
# baseline (speedup 1.0000x reference)
"""Self-contained Bass/Tile TRN2 kernel: 1-layer LSTM encoder over T=20 steps,
batch 65536, hidden 64, data-parallel over batch across 8 NeuronCores.

kernel(**inputs) takes the FULL unsharded inputs (obs_traj [20,65536,2] f32 +
small LSTM/Linear weights) and returns final_h [1, 65536, 64] f32.

Method (per core, batch shard of 8192):
  - Embedding folded into the LSTM input projection on host:
      W_x = W_ih @ W_emb,  bias = b_ih + b_hh + W_ih @ b_emb.
  - All-tanh gate evaluation: weights pre-scaled by 0.5 for the sigmoid
    gates (i,f,o) so one ACT Tanh over the whole [128,2048] PSUM tile gives
    tanh(raw_g) and tanh(raw_ifo/2); sigma = (tanh+1)/2 is recovered with a
    single fused DVE tensor_scalar (add 1, mult 0.5).
  - Batch-major layout: each matmul computes gates for a group of 128 batch
    elements: out[128 elems, 256 gates] = lhsT.T @ rhs with the DATA as the
    stationary operand and the weights streamed.  Per group one K=64 h-part
    matmul (pair-stacked even/odd row halves) opens the accumulation and one
    K=3 const/x matmul (rotated over the four 32-row strips) closes it.
  - The const-1 row rides the per-step x DMA (host prepends a ones row, so
    xt is [T, 3, BC]) -- no per-row memsets.
  - Cell update as bf16 tensor_tensor ops (v = sig_i*tanh_g on GPSIMD;
    u = sig_f*c, c = u+v, h = sig_o*tanh(c) on DVE); tanh(c) batched over 4
    super-chunks per ACT instruction.
  - h returns to the feature-major matmul operand via the DMA xbar
    transpose; h-state is pair-stacked [128, 4096] (even group's features on
    partitions 0:64, odd on 64:128) so each transpose lands exactly on the
    xbar's 128-column blocks.
"""

import numpy as np
import ml_dtypes

import concourse.bass as bass
import concourse.mybir as mybir
import concourse.tile as tile_mod
from concourse.tile import TileContext
import bass_rust as _bass_rust
from bass_rust import ScopedClock, VectorClock
from concourse.tile_scheduler import N_PROCS

BF16 = mybir.dt.bfloat16
F32 = mybir.dt.float32
AluOp = mybir.AluOpType

T_STEPS = 20
B_FULL = 65536
N_CORES = 8
BC = B_FULL // N_CORES          # 8192
HID = 64
NGATE = 256
GROUP = 128
NGROUPS = BC // GROUP           # 64
SUPER = 8                       # groups per super-chunk (one 4-bank psum)
NSUPER = NGROUPS // SUPER       # 8
SCOLS = SUPER * GROUP           # 1024
SPH = 4                         # super-chunks per tanh(c) instruction


def _patched_drain_and_barrier(self, tick_clock, wait_clock):
    # This walrus build accepts at most ONE sync-wait per instruction; the
    # stock tail Drain carries one wait per live proc.  Emit one NOP per
    # proc instead, each carrying a single wait.
    gc = tick_clock.global_clock
    for p in range(N_PROCS):
        t = gc[p]
        if t <= 0:
            continue
        nop = self.nc.sync.nop(nofuse=True, hint=f"tail_wait_p{p}")
        wait_clock.add_sem_waits(
            nop.ins,
            ScopedClock(
                {None: VectorClock([t if q == p else 0 for q in range(N_PROCS)])}
            ),
        )
    self.nc.sync.drain()
    self.nc.all_engine_barrier()
    assert self.sems is not None
    popped = self.nc._tile_sem_poison_stack.pop()
    assert popped is self._sem_poison
    self.nc.clear_and_free_semaphores(list(self.sems.allocated().values()))
    self.nc.all_engine_barrier()


tile_mod.TileContext._drain_and_barrier = _patched_drain_and_barrier


def split_excess_waits(nc, max_waits=1):
    """Hoist excess semaphore waits onto same-engine NOPs placed directly
    before the instruction (the engine blocks at the same point)."""
    ctr = 0
    for fn in nc.m.functions:
        for bb in fn.blocks:
            il = bb.instructions
            i = 0
            while i < len(il):
                inst = il[i]
                si = inst.sync_info
                waits = list(si.on_wait) if si is not None and si.on_wait else []
                if len(waits) > max_waits:
                    sem_waits = [w for w in waits if w.sync_type == "semaphore"]
                    other = [w for w in waits if w.sync_type != "semaphore"]
                    keep_n = max(0, max_waits - len(other))
                    keep = other + sem_waits[:keep_n]
                    extra = sem_waits[keep_n:]
                    pos = i
                    for j in range(0, len(extra), max(1, max_waits)):
                        chunk = extra[j:j + max(1, max_waits)]
                        nop = mybir.InstNoOp(name=f"wsplit-{ctr}", ins=[],
                                             outs=[])
                        ctr += 1
                        nop.engine = inst.engine
                        nop.sync_info = _bass_rust.SyncInfo(
                            on_wait=chunk, on_update=[])
                        il.insert(pos, nop)
                        pos += 1
                        i += 1
                    inst.sync_info = _bass_rust.SyncInfo(
                        on_wait=keep,
                        on_update=list(si.on_update) if si.on_update else [])
                i += 1
    return ctr


def host_weights(W_emb, b_emb, W_ih, W_hh, b_ih, b_hh):
    W_x = W_ih @ W_emb                      # [256, 2]
    bias = b_ih + b_hh + W_ih @ b_emb       # [256]
    colscale = np.concatenate(
        [np.full(64, 0.5), np.full(64, 0.5), np.full(64, 1.0),
         np.full(64, 0.5)]).astype(np.float32)   # i, f, g, o
    Wc = np.zeros((67, NGATE), np.float32)
    Wc[0:64] = W_hh.T * colscale
    Wc[64] = bias * colscale                # const-1 row
    Wc[65:67] = W_x.T * colscale
    return Wc.astype(ml_dtypes.bfloat16)


def build_nc():
    nc = bass.Bass("TRN2", target_bir_lowering=False)
    # xt rows per step: [1, x0, x1] (ones row baked on host)
    xt = nc.dram_tensor("xt", [T_STEPS, 3, BC], BF16, kind="ExternalInput")
    wh = nc.dram_tensor("wh", [64, NGATE], BF16, kind="ExternalInput")
    wxc = nc.dram_tensor("wxc", [3, NGATE], BF16, kind="ExternalInput")
    hout = nc.dram_tensor("hout", [BC, HID], F32, kind="ExternalOutput")

    with TileContext(nc) as tc:
        with (
            tc.tile_pool(name="state", bufs=1) as state_pool,
            tc.tile_pool(name="tpool", bufs=8) as t_pool,
            tc.tile_pool(name="uv", bufs=8) as uv_pool,
            tc.tile_pool(name="tcpool", bufs=4) as tc_pool,
            tc.tile_pool(name="hb", bufs=8) as hb_pool,
            tc.tile_pool(name="psum", bufs=2, space="PSUM") as psum_pool,
        ):
            W2 = state_pool.tile([128, NGATE], BF16, tag="W2")
            nc.sync.dma_start(W2[0:64, :], wh[:])
            nc.sync.dma_start(W2[64:128, :], wh[:])
            # xc weight rows replicated on the four 32-strips
            Wxc = state_pool.tile([128, NGATE], BF16, tag="Wxc")
            for st in (0, 32, 64, 96):
                nc.sync.dma_start(Wxc[st:st + 3, :], wxc[:])

            Xh2a = state_pool.tile([128, BC // 2], BF16, tag="Xh2a")
            Xh2b = state_pool.tile([128, BC // 2], BF16, tag="Xh2b")
            # xc data tile: [1; x0; x1] replicated on the four 32-strips
            xca = state_pool.tile([128, BC], BF16, tag="xca")
            xcb = state_pool.tile([128, BC], BF16, tag="xcb")
            C = state_pool.tile([128, NGROUPS * HID], BF16, tag="C")

            Xh2_of = [Xh2a, Xh2b]
            xc_of = [xca, xcb]
            nc.vector.memset(Xh2a[:, :], 0.0)
            for st in (0, 32, 64, 96):
                nc.sync.dma_start(xca[st:st + 3, :], xt[0])

            for t in range(T_STEPS):
                Xh2 = Xh2_of[t % 2]
                Xh2n = Xh2_of[(t + 1) % 2]
                xc = xc_of[t % 2]

                Tts = {}
                for s in range(NSUPER):
                    base = s * (SCOLS // 2)
                    Tt = t_pool.tile([128, SUPER * NGATE], BF16, tag="T")
                    # One PSUM bank per group (cols j*512, 256 used); per
                    # half-super emit the 4 h-MMs as a back-to-back run
                    # (alternating row halves -> concurrent streams), then
                    # the 4 xc-MMs as their own run.  One start per bank.
                    for half in range(2):
                        ph = psum_pool.tile([128, 4 * 512], F32, tag="ps",
                                            name="php")
                        for j in range(4):
                            gl = half * 4 + j
                            par = gl % 2
                            b = gl // 2
                            nc.tensor.matmul(
                                ph[:, j * 512:j * 512 + NGATE],
                                lhsT=Xh2[64 * par:64 * (par + 1),
                                         base + b * GROUP:
                                         base + (b + 1) * GROUP],
                                rhs=W2[64 * par:64 * (par + 1), :],
                                start=True, stop=False)
                        for j in range(4):
                            gl = half * 4 + j
                            st = (gl % 4) * 32
                            g = s * SUPER + gl
                            nc.tensor.matmul(
                                ph[:, j * 512:j * 512 + NGATE],
                                lhsT=xc[st:st + 3, g * GROUP:(g + 1) * GROUP],
                                rhs=Wxc[st:st + 3, :],
                                start=False, stop=True,
                                tile_position=(st, 0))
                        src = ph[:].rearrange(
                            "p (j c) -> p j c", c=512)[:, :, 0:NGATE]
                        nc.scalar.activation(
                            Tt[:, half * 1024:(half + 1) * 1024].rearrange(
                                "p (j c) -> p j c", c=NGATE),
                            src, mybir.ActivationFunctionType.Tanh)
                    Tts[s] = Tt

                    Tg3 = Tt[:].rearrange("p (g c) -> p g c", c=NGATE)
                    Sif = Tg3[:, :, 0:128]
                    So = Tg3[:, :, 192:256]
                    nc.vector.tensor_scalar(
                        Sif, Sif, 1.0, 0.5, AluOp.add, AluOp.mult)
                    nc.vector.tensor_scalar(
                        So, So, 1.0, 0.5, AluOp.add, AluOp.mult)
                    Si = Tg3[:, :, 0:64]
                    Sf = Tg3[:, :, 64:128]
                    Tg = Tg3[:, :, 128:192]
                    So = Tg3[:, :, 192:256]
                    Cs3 = C[:, s * SCOLS // 2:(s + 1) * SCOLS // 2].rearrange(
                        "p (g c) -> p g c", c=HID)
                    if t == 0:
                        nc.gpsimd.tensor_tensor(Cs3, Si, Tg, AluOp.mult)
                    else:
                        u = uv_pool.tile([128, SUPER * HID], BF16, tag="u")
                        v = uv_pool.tile([128, SUPER * HID], BF16, tag="v")
                        u3 = u[:].rearrange("p (g c) -> p g c", c=HID)
                        v3 = v[:].rearrange("p (g c) -> p g c", c=HID)
                        nc.gpsimd.tensor_tensor(v3, Si, Tg, AluOp.mult)
                        nc.vector.tensor_tensor(u3, Sf, Cs3, AluOp.mult)
                        nc.vector.tensor_tensor(Cs3, u3, v3, AluOp.add)

                    if s % SPH == SPH - 1:
                        lo = s - SPH + 1
                        tcols = SPH * SCOLS // 2
                        tcv = tc_pool.tile([128, tcols], BF16, tag="tc")
                        nc.scalar.activation(
                            tcv[:],
                            C[:, lo * SCOLS // 2:lo * SCOLS // 2 + tcols],
                            mybir.ActivationFunctionType.Tanh)
                        for s2 in range(lo, s + 1):
                            tcs3 = tcv[:].rearrange(
                                "p (g c) -> p g c", c=HID)[
                                :, (s2 - lo) * SUPER:(s2 - lo + 1) * SUPER, :]
                            To2 = Tts[s2][:].rearrange(
                                "p (g c) -> p g c", c=NGATE)[:, :, 192:256]
                            hb = hb_pool.tile([128, SUPER * HID], BF16,
                                              tag="hb")
                            hb3 = hb[:].rearrange("p (g c) -> p g c", c=HID)
                            nc.vector.tensor_tensor(hb3, To2, tcs3,
                                                    AluOp.mult)
                            if t < T_STEPS - 1:
                                xdst = Xh2n[
                                    :, s2 * (SCOLS // 2):
                                    (s2 + 1) * (SCOLS // 2)].rearrange(
                                    "p (b e) -> p b e", e=GROUP)
                                nc.sync.dma_start_transpose(xdst, hb[:])
                            else:
                                hf = hb_pool.tile([128, SUPER * HID], F32,
                                                  tag="hf")
                                nc.vector.tensor_copy(hf[:], hb[:])
                                hdst = hout[s2 * SCOLS:(s2 + 1) * SCOLS,
                                            :].rearrange(
                                    "(g e) f -> e g f", e=GROUP)
                                nc.sync.dma_start(
                                    hdst,
                                    hf[:].rearrange("p (g c) -> p g c",
                                                    c=HID))
                    if s == 0 and t < T_STEPS - 1:
                        xcn = xc_of[(t + 1) % 2]
                        for st in (0, 32, 64, 96):
                            nc.sync.dma_start(xcn[st:st + 3, :], xt[t + 1])
    split_excess_waits(nc)
    return nc


_NC_CACHE = {}


def kernel(obs_traj, W_emb, b_emb, W_ih, W_hh, b_ih, b_hh):
    from concourse.bass_utils import run_bass_kernel_spmd

    Wc = host_weights(
        np.asarray(W_emb, dtype=np.float32),
        np.asarray(b_emb, dtype=np.float32),
        np.asarray(W_ih, dtype=np.float32),
        np.asarray(W_hh, dtype=np.float32),
        np.asarray(b_ih, dtype=np.float32),
        np.asarray(b_hh, dtype=np.float32))
    wh = np.ascontiguousarray(Wc[0:64])
    wxc = np.ascontiguousarray(Wc[64:67])
    obs = np.asarray(obs_traj)
    in_maps = []
    for c in range(N_CORES):
        sl = obs[:, c * BC:(c + 1) * BC, :]          # [T, BC, 2]
        xT = np.empty((T_STEPS, 3, BC), np.float32)
        xT[:, 0, :] = 1.0
        xT[:, 1:3, :] = sl.transpose(0, 2, 1)
        in_maps.append({"xt": xT.astype(ml_dtypes.bfloat16),
                        "wh": wh, "wxc": wxc})
    if "nc" not in _NC_CACHE:
        _NC_CACHE["nc"] = build_nc()
    res = run_bass_kernel_spmd(_NC_CACHE["nc"], in_maps,
                               core_ids=list(range(N_CORES)))
    h = np.concatenate([r["hout"] for r in res.results], axis=0)
    return h[None].astype(np.float32)



# revision 2
# speedup vs baseline: 1.1951x; 1.1951x over previous
"""Self-contained Bass/Tile TRN2 kernel: 1-layer LSTM encoder over T=20 steps,
batch 65536, hidden 64, data-parallel over batch across 8 NeuronCores.

kernel(**inputs) takes the FULL unsharded inputs (obs_traj [20,65536,2] f32 +
small LSTM/Linear weights) and returns final_h [1, 65536, 64] f32.

Method (per core, batch shard of 8192):
  - Embedding folded into the LSTM input projection on host:
      W_x = W_ih @ W_emb,  bias = b_ih + b_hh + W_ih @ b_emb.
  - All-tanh gate evaluation: weights pre-scaled by 0.5 for the sigmoid
    gates (i,f,o) so one ACT Tanh over the whole [128,2048] PSUM tile gives
    tanh(raw_g) and tanh(raw_ifo/2); sigma = (tanh+1)/2 is recovered with a
    single fused DVE tensor_scalar (add 1, mult 0.5).
  - Batch-major layout: each matmul computes gates for a group of 128 batch
    elements: out[128 elems, 256 gates] = lhsT.T @ rhs with the DATA as the
    stationary operand and the weights streamed.  Per group one K=64 h-part
    matmul (pair-stacked even/odd row halves) opens the accumulation and one
    K=3 const/x matmul (rotated over the four 32-row strips) closes it.
  - The const-1 row rides the per-step x DMA (host prepends a ones row, so
    xt is [T, 3, BC]) -- no per-row memsets.
  - Cell update as bf16 tensor_tensor ops (v = sig_i*tanh_g on GPSIMD;
    u = sig_f*c, c = u+v, h = sig_o*tanh(c) on DVE); tanh(c) batched over 4
    super-chunks per ACT instruction.
  - h returns to the feature-major matmul operand via the DMA xbar
    transpose; h-state is pair-stacked [128, 4096] (even group's features on
    partitions 0:64, odd on 64:128) so each transpose lands exactly on the
    xbar's 128-column blocks.
"""

import numpy as np
import ml_dtypes

import concourse.bass as bass
import concourse.mybir as mybir
import concourse.tile as tile_mod
from concourse.tile import TileContext
import bass_rust as _bass_rust
from bass_rust import ScopedClock, VectorClock
from concourse.tile_scheduler import N_PROCS

BF16 = mybir.dt.bfloat16
F32 = mybir.dt.float32
AluOp = mybir.AluOpType

T_STEPS = 20
B_FULL = 65536
N_CORES = 8
BC = B_FULL // N_CORES          # 8192
HID = 64
NGATE = 256
GROUP = 128
NGROUPS = BC // GROUP           # 64
SUPER = 8                       # groups per super-chunk (one 4-bank psum)
NSUPER = NGROUPS // SUPER       # 8
SCOLS = SUPER * GROUP           # 1024
SPH = 4                         # super-chunks per tanh(c) instruction


def _patched_drain_and_barrier(self, tick_clock, wait_clock):
    # This walrus build accepts at most ONE sync-wait per instruction; the
    # stock tail Drain carries one wait per live proc.  Emit one NOP per
    # proc instead, each carrying a single wait.
    gc = tick_clock.global_clock
    for p in range(N_PROCS):
        t = gc[p]
        if t <= 0:
            continue
        nop = self.nc.sync.nop(nofuse=True, hint=f"tail_wait_p{p}")
        wait_clock.add_sem_waits(
            nop.ins,
            ScopedClock(
                {None: VectorClock([t if q == p else 0 for q in range(N_PROCS)])}
            ),
        )
    self.nc.sync.drain()
    self.nc.all_engine_barrier()
    assert self.sems is not None
    popped = self.nc._tile_sem_poison_stack.pop()
    assert popped is self._sem_poison
    self.nc.clear_and_free_semaphores(list(self.sems.allocated().values()))
    self.nc.all_engine_barrier()


tile_mod.TileContext._drain_and_barrier = _patched_drain_and_barrier


def split_excess_waits(nc, max_waits=1):
    """Hoist excess semaphore waits onto same-engine NOPs placed directly
    before the instruction (the engine blocks at the same point)."""
    ctr = 0
    for fn in nc.m.functions:
        for bb in fn.blocks:
            il = bb.instructions
            i = 0
            while i < len(il):
                inst = il[i]
                si = inst.sync_info
                waits = list(si.on_wait) if si is not None and si.on_wait else []
                if len(waits) > max_waits:
                    sem_waits = [w for w in waits if w.sync_type == "semaphore"]
                    other = [w for w in waits if w.sync_type != "semaphore"]
                    keep_n = max(0, max_waits - len(other))
                    keep = other + sem_waits[:keep_n]
                    extra = sem_waits[keep_n:]
                    pos = i
                    for j in range(0, len(extra), max(1, max_waits)):
                        chunk = extra[j:j + max(1, max_waits)]
                        nop = mybir.InstNoOp(name=f"wsplit-{ctr}", ins=[],
                                             outs=[])
                        ctr += 1
                        nop.engine = inst.engine
                        nop.sync_info = _bass_rust.SyncInfo(
                            on_wait=chunk, on_update=[])
                        il.insert(pos, nop)
                        pos += 1
                        i += 1
                    inst.sync_info = _bass_rust.SyncInfo(
                        on_wait=keep,
                        on_update=list(si.on_update) if si.on_update else [])
                i += 1
    return ctr


def host_weights(W_emb, b_emb, W_ih, W_hh, b_ih, b_hh):
    W_x = W_ih @ W_emb                      # [256, 2]
    bias = b_ih + b_hh + W_ih @ b_emb       # [256]
    colscale = np.concatenate(
        [np.full(64, 0.5), np.full(64, 0.5), np.full(64, 1.0),
         np.full(64, 0.5)]).astype(np.float32)   # i, f, g, o
    Wc = np.zeros((67, NGATE), np.float32)
    Wc[0:64] = W_hh.T * colscale
    Wc[64] = bias * colscale                # const-1 row
    Wc[65:67] = W_x.T * colscale
    return Wc.astype(ml_dtypes.bfloat16)


def build_nc():
    nc = bass.Bass("TRN2", target_bir_lowering=False)
    # xt rows per step: [1, x0, x1] (ones row baked on host)
    xt = nc.dram_tensor("xt", [T_STEPS, 3, BC], BF16, kind="ExternalInput")
    wh = nc.dram_tensor("wh", [64, NGATE], BF16, kind="ExternalInput")
    wxc = nc.dram_tensor("wxc", [3, NGATE], BF16, kind="ExternalInput")
    hout = nc.dram_tensor("hout", [BC, HID], F32, kind="ExternalOutput")

    with TileContext(nc) as tc:
        with (
            tc.tile_pool(name="state", bufs=1) as state_pool,
            tc.tile_pool(name="tpool", bufs=8) as t_pool,
            tc.tile_pool(name="uv", bufs=8) as uv_pool,
            tc.tile_pool(name="tcpool", bufs=4) as tc_pool,
            tc.tile_pool(name="hb", bufs=8) as hb_pool,
            tc.tile_pool(name="psum", bufs=2, space="PSUM") as psum_pool,
        ):
            W2 = state_pool.tile([128, NGATE], BF16, tag="W2")
            nc.sync.dma_start(W2[0:64, :], wh[:])
            nc.sync.dma_start(W2[64:128, :], wh[:])
            # xc weight rows replicated on the four 32-strips
            Wxc = state_pool.tile([128, NGATE], BF16, tag="Wxc")
            for st in (0, 32, 64, 96):
                nc.sync.dma_start(Wxc[st:st + 3, :], wxc[:])

            Xh2a = state_pool.tile([128, BC // 2], BF16, tag="Xh2a")
            Xh2b = state_pool.tile([128, BC // 2], BF16, tag="Xh2b")
            # xc data tile: [1; x0; x1] replicated on the four 32-strips
            xca = state_pool.tile([128, BC], BF16, tag="xca")
            xcb = state_pool.tile([128, BC], BF16, tag="xcb")
            C = state_pool.tile([128, NGROUPS * HID], BF16, tag="C")

            Xh2_of = [Xh2a, Xh2b]
            xc_of = [xca, xcb]
            nc.vector.memset(Xh2a[:, :], 0.0)
            for st in (0, 32, 64, 96):
                nc.sync.dma_start(xca[st:st + 3, :], xt[0])

            for t in range(T_STEPS):
                Xh2 = Xh2_of[t % 2]
                Xh2n = Xh2_of[(t + 1) % 2]
                xc = xc_of[t % 2]

                Tts = {}
                for s in range(NSUPER):
                    base = s * (SCOLS // 2)
                    Tt = t_pool.tile([128, SUPER * NGATE], BF16, tag="T")
                    # One PSUM bank per group (cols j*512, 256 used); per
                    # half-super emit the 4 h-MMs as a back-to-back run
                    # (alternating row halves -> concurrent streams), then
                    # the 4 xc-MMs as their own run.  One start per bank.
                    for half in range(2):
                        ph = psum_pool.tile([128, 4 * 512], F32, tag="ps",
                                            name="php")
                        for j in range(4):
                            gl = half * 4 + j
                            par = gl % 2
                            b = gl // 2
                            nc.tensor.matmul(
                                ph[:, j * 512:j * 512 + NGATE],
                                lhsT=Xh2[64 * par:64 * (par + 1),
                                         base + b * GROUP:
                                         base + (b + 1) * GROUP],
                                rhs=W2[64 * par:64 * (par + 1), :],
                                start=True, stop=False)
                        for j in range(4):
                            gl = half * 4 + j
                            st = (gl % 4) * 32
                            g = s * SUPER + gl
                            nc.tensor.matmul(
                                ph[:, j * 512:j * 512 + NGATE],
                                lhsT=xc[st:st + 3, g * GROUP:(g + 1) * GROUP],
                                rhs=Wxc[st:st + 3, :],
                                start=False, stop=True,
                                tile_position=(st, 0))
                        src = ph[:].rearrange(
                            "p (j c) -> p j c", c=512)[:, :, 0:NGATE]
                        nc.scalar.activation(
                            Tt[:, half * 1024:(half + 1) * 1024].rearrange(
                                "p (j c) -> p j c", c=NGATE),
                            src, mybir.ActivationFunctionType.Tanh)
                    Tts[s] = Tt

                    Tg3 = Tt[:].rearrange("p (g c) -> p g c", c=NGATE)
                    Sif = Tg3[:, :, 0:128]
                    So = Tg3[:, :, 192:256]
                    nc.vector.tensor_scalar(
                        Sif, Sif, 1.0, 0.5, AluOp.add, AluOp.mult)
                    nc.vector.tensor_scalar(
                        So, So, 1.0, 0.5, AluOp.add, AluOp.mult)
                    Si = Tg3[:, :, 0:64]
                    Sf = Tg3[:, :, 64:128]
                    Tg = Tg3[:, :, 128:192]
                    So = Tg3[:, :, 192:256]
                    Cs3 = C[:, s * SCOLS // 2:(s + 1) * SCOLS // 2].rearrange(
                        "p (g c) -> p g c", c=HID)
                    if t == 0:
                        nc.gpsimd.tensor_tensor(Cs3, Si, Tg, AluOp.mult)
                    else:
                        u = uv_pool.tile([128, SUPER * HID], BF16, tag="u")
                        v = uv_pool.tile([128, SUPER * HID], BF16, tag="v")
                        u3 = u[:].rearrange("p (g c) -> p g c", c=HID)
                        v3 = v[:].rearrange("p (g c) -> p g c", c=HID)
                        nc.gpsimd.tensor_tensor(v3, Si, Tg, AluOp.mult)
                        nc.vector.tensor_tensor(u3, Sf, Cs3, AluOp.mult)
                        nc.vector.tensor_tensor(Cs3, u3, v3, AluOp.add)

                    if s % SPH == SPH - 1:
                        lo = s - SPH + 1
                        tcols = SPH * SCOLS // 2
                        tcv = tc_pool.tile([128, tcols], BF16, tag="tc")
                        nc.scalar.activation(
                            tcv[:],
                            C[:, lo * SCOLS // 2:lo * SCOLS // 2 + tcols],
                            mybir.ActivationFunctionType.Tanh)
                        for s2 in range(lo, s + 1):
                            tcs3 = tcv[:].rearrange(
                                "p (g c) -> p g c", c=HID)[
                                :, (s2 - lo) * SUPER:(s2 - lo + 1) * SUPER, :]
                            To2 = Tts[s2][:].rearrange(
                                "p (g c) -> p g c", c=NGATE)[:, :, 192:256]
                            hb = hb_pool.tile([128, SUPER * HID], BF16,
                                              tag="hb")
                            hb3 = hb[:].rearrange("p (g c) -> p g c", c=HID)
                            nc.vector.tensor_tensor(hb3, To2, tcs3,
                                                    AluOp.mult)
                            if t < T_STEPS - 1:
                                xdst = Xh2n[
                                    :, s2 * (SCOLS // 2):
                                    (s2 + 1) * (SCOLS // 2)].rearrange(
                                    "p (b e) -> p b e", e=GROUP)
                                nc.sync.dma_start_transpose(xdst, hb[:])
                            else:
                                hf = hb_pool.tile([128, SUPER * HID], F32,
                                                  tag="hf")
                                nc.vector.tensor_copy(hf[:], hb[:])
                                hdst = hout[s2 * SCOLS:(s2 + 1) * SCOLS,
                                            :].rearrange(
                                    "(g e) f -> e g f", e=GROUP)
                                nc.sync.dma_start(
                                    hdst,
                                    hf[:].rearrange("p (g c) -> p g c",
                                                    c=HID))
                    if s == 0 and t < T_STEPS - 1:
                        xcn = xc_of[(t + 1) % 2]
                        for st in (0, 32, 64, 96):
                            nc.sync.dma_start(xcn[st:st + 3, :], xt[t + 1])
    split_excess_waits(nc)
    return nc


_NC_CACHE = {}


def host_inputs(obs, Wc):
    wh = np.ascontiguousarray(Wc[0:64])
    wxc = np.ascontiguousarray(Wc[64:67])
    obs = np.asarray(obs)
    in_maps = []
    for c in range(N_CORES):
        sl = obs[:, c * BC:(c + 1) * BC, :]          # [T, BC, 2]
        xT = np.empty((T_STEPS, 3, BC), np.float32)
        xT[:, 0, :] = 1.0
        xT[:, 1:3, :] = sl.transpose(0, 2, 1)
        in_maps.append({"xt": xT.astype(ml_dtypes.bfloat16),
                        "wh": wh, "wxc": wxc})
    return in_maps


def host_gather(res):
    h = np.concatenate([r["hout"] for r in res.results], axis=0)
    return h[None].astype(np.float32)


def kernel(obs_traj, W_emb, b_emb, W_ih, W_hh, b_ih, b_hh):
    from concourse.bass_utils import run_bass_kernel_spmd

    Wc = host_weights(
        np.asarray(W_emb, dtype=np.float32),
        np.asarray(b_emb, dtype=np.float32),
        np.asarray(W_ih, dtype=np.float32),
        np.asarray(W_hh, dtype=np.float32),
        np.asarray(b_ih, dtype=np.float32),
        np.asarray(b_hh, dtype=np.float32))
    in_maps = host_inputs(obs_traj, Wc)
    if "nc" not in _NC_CACHE:
        _NC_CACHE["nc"] = build_nc()
    res = run_bass_kernel_spmd(_NC_CACHE["nc"], in_maps,
                               core_ids=list(range(N_CORES)))
    return host_gather(res)

